# revision 24
# baseline (speedup 1.0000x reference)
"""Trainium2 Bass kernel for nn_Caps2dMatwo (capsule conv + matwo dual routing).

Sharding: 8 cores = (batch n: 4) x (h-half: 2); each core computes a 48-row
slab of one batch element independently (halo via host padding, no collectives).

v4: u_hat column order m' = (co', a, b, tp) - channel-pure 32-blocks keep the
tile-position transform matmuls valid while making tp (thus t = 2co'+tp)
innermost, so every routing broadcast runs the DVE in 2x mode. Uneven chunks
(3,9,9,9,6 blocks) shrink the pipeline fill/tail. Sigmoid (clamped Pade tanh)
and rsqrt (Quake seed + Newton) run on the DVE so the ACT engine never swaps
function tables and the routing chain never round-trips through ACT. The
app-squash n2 comes from ar (n2 = sum_i r_i*ar_app,i), saving a square+reduce
on iters 1-2. App bias rides the ACT drain (Identity + per-partition bias).
"""
import sys
import numpy as np

sys.path.insert(0, "/opt/trn_rl_repo")

import concourse.bass as bass
import concourse.bacc as bacc
import concourse.mybir as mybir
from concourse import tile
from concourse.bass_utils import run_bass_kernel_spmd
import ml_dtypes

BF16 = mybir.dt.float16
F32 = mybir.dt.float32
AL = mybir.AluOpType
AF = mybir.ActivationFunctionType
AX = mybir.AxisListType

T0, T1, Z, H, W, HC = 4, 8, 32, 96, 96, 48
NBLK = 36
CHUNKS = [(0, 3), (3, 9), (12, 9), (21, 9), (30, 6)]


# ----------------------------------------------------------------------------
# host-side weight/layout construction
# ----------------------------------------------------------------------------

def _build_weights(W_conv, W_pos, W_app, b_app):
    CW = np.zeros((96, T0, 32, 8), np.float32)
    for hi in range(8):
        for wi in range(12):
            for pi in range(4):
                for pj in range(8):
                    dy, dx = hi - pi, wi - pj
                    if 0 <= dy < 5 and 0 <= dx < 5:
                        CW[hi * 12 + wi, :, pi * 8 + pj, :] = W_conv[:, dy, dx, 0, :]

    m_pos = np.stack([W_pos[i].reshape(T1, 4, 4) for i in range(T0)])
    m_app = np.stack([W_app[i].reshape(T1, 4, 4) for i in range(T0)])
    nrm = np.sqrt(np.maximum((m_pos ** 2).sum(axis=2, keepdims=True), 1e-12))
    m_pos = m_pos / nrm

    # m' = co*32 + a*8 + b*2 + tp ; contraction row z = 16tp + 4a + c
    TW = np.zeros((128, T0, 2, 128), np.float32)
    for i in range(T0):
        blkp = np.zeros((32, 128), np.float32)
        blka = np.zeros((32, 128), np.float32)
        for co in range(4):
            for tp in range(2):
                t = 2 * co + tp
                for a in range(4):
                    for b in range(4):
                        m = co * 32 + a * 8 + b * 2 + tp
                        for c in range(4):
                            z = 16 * tp + 4 * a + c
                            blkp[z, m] = m_pos[i, t, c, b]
                            blka[z, m] = m_app[i, t, c, b]
        for j in range(4):
            TW[32 * j:32 * j + 32, i, 0] = blkp
            TW[32 * j:32 * j + 32, i, 1] = blka

    # raw extract: m'' = co*8 + a*2 + tp, one selector block per channel cp
    RW3 = np.zeros((128, 4, 32), np.float32)
    for cp in range(4):
        for tp in range(2):
            for a in range(4):
                z = 16 * tp + 4 * a + 3
                for j in range(4):
                    RW3[32 * j + z, cp, cp * 8 + a * 2 + tp] = 1.0

    KA = np.zeros((128, T0), np.float32)
    for i in range(T0):
        for co in range(4):
            for tp in range(2):
                t = 2 * co + tp
                for a in range(4):
                    for b in range(4):
                        m = co * 32 + a * 8 + b * 2 + tp
                        KA[m, i] = b_app[i, t] * m_app[i, t, :, b].sum()
    return CW, TW, RW3, KA


_PH = np.arange(NBLK) // 3
_B3 = np.arange(NBLK) % 3
_HIDX = (4 * _PH)[:, None] + np.arange(8)[None, :]
_PWJ = (4 * _B3)[:, None] + np.arange(4)[None, :]
_WIDX = (8 * _PWJ)[:, :, None] + np.arange(12)[None, None, :]


def _build_patches(pad):
    g = pad[:, :, _HIDX[:, None, :, None], _WIDX[:, :, None, :]]
    return np.ascontiguousarray(
        g.transpose(4, 5, 0, 2, 3, 1).reshape(96, T0, NBLK, 4, Z))


def _pixel_coords(hh):
    xs = np.zeros((128, NBLK, 2), np.float32)
    for b in range(NBLK):
        ph, b3 = b // 3, b % 3
        for j in range(4):
            for pi in range(4):
                for pj in range(8):
                    part = j * 32 + pi * 8 + pj
                    xs[part, b, 0] = (8 * (4 * b3 + j) + pj) / W
                    xs[part, b, 1] = (4 * ph + pi + 48 * hh) / H
    # pre-broadcast over m''=(co,a,tp) so the coord-add multiply is packed bf16
    return np.ascontiguousarray(
        np.broadcast_to(xs[:, :, :, None], (128, NBLK, 2, 32)))


# ----------------------------------------------------------------------------
# device kernel
# ----------------------------------------------------------------------------

def _routing_chunk(nc, mpool, rpool, U, rawt, xy, s0, S):
    """U: ubig chunk view [128, S, T0, 2, 128(m'=co,a,b,tp)];
    rawt [128, S, T0, 32(m''=co,a,tp)]; xy [128, NBLK, 2, 32] bf16."""
    Uf = U.rearrange("p s i pa c -> p s i (pa c)")        # [128, S, 4, 256]

    # ---- coordinate addition -------------------------------------------
    # U[..., pa=0, co, a, b=k, tp] += xy_k * raw[co, a, tp]
    PB = 1 if S >= 6 else 0          # trailing blocks co-issued on GPSIMD
    SD = S - PB

    def co_tt(out, in0, in1, op, sdim):
        """Co-issue a tensor_tensor: leading SD blocks on DVE, trailing PB
        on the (otherwise idle) GPSIMD. sdim = index of the s/si dim."""
        cut = (SD * out.shape[sdim]) // S
        sl_d = tuple([slice(None)] * sdim + [slice(0, cut)])
        sl_p = tuple([slice(None)] * sdim + [slice(cut, out.shape[sdim])])
        nc.vector.tensor_tensor(out[sl_d], in0[sl_d], in1[sl_d], op=op)
        if PB:
            nc.gpsimd.tensor_tensor(out[sl_p], in0[sl_p], in1[sl_p], op=op)

    tmpc = mpool.tile([128, 9, T0, 32], BF16, name="tmpc", tag="tmpc", bufs=1)[:, :S]
    Ub = U.rearrange("p s i pa (ca b tp) -> p (s i) pa ca b tp", b=4, tp=2)
    tcb = tmpc.rearrange("p s i (ca tp) -> p (s i) ca tp", tp=2)
    for k in range(2):
        xyb = xy[:, s0:s0 + S, k].unsqueeze(2).broadcast_to([128, S, T0, 32])
        co_tt(tmpc, rawt, xyb, AL.mult, 1)
        usl = Ub[:, :, 0, :, k, :]
        co_tt(usl, tcb, usl, AL.add, 1)

    # ---- p = sum_i U_i (unscaled; r=0.5 folded into stats scalings) ----
    p = rpool.tile([128, 9, 2, 128], BF16, name="p", tag="p", bufs=2)[:, :S]
    ts1 = rpool.tile([128, 9, 256], BF16, name="ts1", tag="ts", bufs=2)[:, :S]
    ts2 = rpool.tile([128, 9, 256], BF16, name="ts2", tag="ts", bufs=2)[:, :S]
    pf = p.rearrange("p s pa c -> p s (pa c)")
    co_tt(ts1, Uf[:, :, 0], Uf[:, :, 1], AL.add, 1)
    co_tt(ts2, Uf[:, :, 2], Uf[:, :, 3], AL.add, 1)
    co_tt(pf, ts1, ts2, AL.add, 1)

    def statP(tag):
        # psquash scale sfp = 1/max_z|p_pos| per t=(co,tp)
        ppos = p[:, :, 0].rearrange("p s (co a b tp) -> p s co tp (a b)",
                                    co=4, a=4, b=4)
        m = mpool.tile([128, 9, 4, 2], F32, name=f"m{tag}", tag="st_m")
        nc.vector.tensor_reduce(m[:, :S], ppos, axis=AX.X, op=AL.max,
                                apply_absolute_value=True)
        sfpf = mpool.tile([128, 9, 8], F32, name=f"sfpf{tag}", tag="st_sfpf")
        nc.vector.reciprocal_approx_fast(
            sfpf[:, :S].rearrange("p s c -> p (s c)"),
            m[:, :S].rearrange("p s c t -> p (s c t)"))
        sfp = mpool.tile([128, 9, 8], BF16, name=f"sfp{tag}", tag=f"sfp{tag}",
                         bufs=1)
        nc.scalar.copy(sfp[:, :S], sfpf[:, :S])
        return sfp[:, :S]

    def statA(n2sum, scale_n2, scale_a, tag):
        # sfa = n2*rsqrt(n2+eps)*scale_a/(1+n2), n2 = scale_n2*n2sum;
        # rsqrt via Quake seed + one Newton step (all on the DVE).
        n2f = n2sum.rearrange("p s c t -> p (s c t)")
        nsq = mpool.tile([128, 9, 8], F32, name=f"nsq{tag}", tag="st_nsq")
        u = nsq[:, :S].rearrange("p s c -> p (s c)")
        nc.vector.tensor_scalar(u, n2f, scale_n2, 1e-9, op0=AL.mult, op1=AL.add)
        y0t = mpool.tile([128, 9, 8], F32, name=f"y0{tag}", tag="st_y0")
        y0 = y0t[:, :S].rearrange("p s c -> p (s c)")
        nc.vector.tensor_scalar(y0.bitcast(mybir.dt.int32),
                                u.bitcast(mybir.dt.int32), 1, None,
                                op0=AL.logical_shift_right)
        # 0x5f3759df - y == (y ^ -1) + 0x5f3759e0 (two's complement)
        nc.vector.tensor_scalar(y0.bitcast(mybir.dt.int32),
                                y0.bitcast(mybir.dt.int32), -1, None,
                                op0=AL.bitwise_xor)
        nc.vector.tensor_scalar(y0.bitcast(mybir.dt.int32),
                                y0.bitcast(mybir.dt.int32), 0x5f3759e0, None,
                                op0=AL.add)
        ht = mpool.tile([128, 9, 8], F32, name=f"h{tag}", tag="st_h")
        h = ht[:, :S].rearrange("p s c -> p (s c)")
        nc.vector.tensor_tensor(h, y0, y0, op=AL.mult)
        nc.vector.tensor_tensor(h, h, u, op=AL.mult)
        nc.vector.tensor_scalar(h, h, -0.5, 1.5, op0=AL.mult, op1=AL.add)
        nc.vector.tensor_tensor(y0, y0, h, op=AL.mult)   # y0 = rsqrt(u)
        den = mpool.tile([128, 9, 8], F32, name=f"den{tag}", tag="st_den")
        dnf = den[:, :S].rearrange("p s c -> p (s c)")
        nc.vector.tensor_scalar(dnf, n2f, scale_n2 / scale_a, 1.0 / scale_a,
                                op0=AL.mult, op1=AL.add)
        rec = mpool.tile([128, 9, 8], F32, name=f"rec{tag}", tag="st_rec")
        rcf = rec[:, :S].rearrange("p s c -> p (s c)")
        nc.vector.reciprocal_approx_fast(rcf, dnf)
        nc.vector.tensor_tensor(y0, y0, rcf, op=AL.mult)
        sfa = mpool.tile([128, 9, 8], BF16, name=f"sfa{tag}", tag=f"sfa{tag}",
                         bufs=1)
        nc.vector.scalar_tensor_tensor(
            sfa[:, :S].rearrange("p s c -> p (s c)"), n2f, scale_n2, y0,
            op0=AL.mult, op1=AL.mult)
        return sfa[:, :S]

    def sigmoid_dve(bacc, name):
        # sigmoid(b) ~= 0.5 + 0.5*pade_tanh(clamp(b/2, +-3)); |err| < 0.005
        bf = bacc.rearrange("p s i c -> p (s i c)")
        xt = mpool.tile([128, 9, T0, 8], F32, name=f"x{name}", tag="sg_x", bufs=1)
        x = xt[:, :S].rearrange("p s i c -> p (s i c)")
        nc.vector.tensor_scalar(x, bf, 0.5, None, op0=AL.mult)
        nc.vector.tensor_scalar(x, x, -3.0, 3.0, op0=AL.max, op1=AL.min)
        dt_ = mpool.tile([128, 9, T0, 8], F32, name=f"d{name}", tag="sg_d", bufs=1)
        dd = dt_[:, :S].rearrange("p s i c -> p (s i c)")
        nc.vector.tensor_tensor(dd, x, x, op=AL.mult)     # x^2
        tt = mpool.tile([128, 9, T0, 8], F32, name=f"t{name}", tag="sg_t", bufs=1)
        t = tt[:, :S].rearrange("p s i c -> p (s i c)")
        nc.vector.scalar_tensor_tensor(t, dd, 27.0, x, op0=AL.add, op1=AL.mult)
        nc.vector.tensor_scalar(dd, dd, 9.0, 27.0, op0=AL.mult, op1=AL.add)
        nc.vector.reciprocal_approx_fast(dd, dd)
        nc.vector.tensor_tensor(t, t, dd, op=AL.mult)
        r = rpool.tile([128, 9, T0, 8], BF16, name=name, tag="r2", bufs=2)
        nc.vector.tensor_scalar(r[:, :S].rearrange("p s i c -> p (s i c)"),
                                t, 0.5, 0.5, op0=AL.mult, op1=AL.add)
        return r[:, :S]

    w = rpool.tile([128, 9, T0, 256], BF16, name="w", tag="w", bufs=2)[:, :S]
    wpa = w.rearrange("p s i (pa co ab tp) -> p (s i) pa (co ab tp)",
                      pa=2, co=4, tp=2)

    def araw(tag):
        """w holds U*p'; reduce z=(a,b) -> ar [128, 8S(sipa), 4co, 2tp].
        b-level-1 is in place in w (w is consumed; next mult rewrites it)."""
        wz = w.rearrange("p s i (pa ca b tp) -> p (s i) pa ca b tp",
                         pa=2, b=4, tp=2)
        for pa in range(2):          # b: 4 -> 2, in place into b0:2
            co_tt(wz[:, :, pa, :, 0:2], wz[:, :, pa, :, 0:2],
                  wz[:, :, pa, :, 2:4], AL.add, 1)
        t2 = rpool.tile([128, 36, 2, 16, 2], BF16, name=f"t2{tag}",
                        tag="t2", bufs=1)[:, :4 * S]
        for pa in range(2):          # b: 2 -> 1
            co_tt(t2[:, :, pa], wz[:, :, pa, :, 0], wz[:, :, pa, :, 1],
                  AL.add, 1)
        t2v = t2.rearrange("p si pa (co a) tp -> p (si pa) co a tp", a=4)
        t3 = rpool.tile([128, 72, 4, 2, 2], BF16, name=f"t3{tag}",
                        tag="t3", bufs=1)[:, :8 * S]
        nc.vector.tensor_tensor(t3, t2v[:, :, :, 0:2], t2v[:, :, :, 2:4],
                                op=AL.add)
        ar = rpool.tile([128, 72, 4, 2], BF16, name=f"ar{tag}", tag="ar",
                        bufs=2)[:, :8 * S]
        nc.vector.tensor_tensor(ar, t3[:, :, :, 0], t3[:, :, :, 1],
                                op=AL.add)
        return ar

    def arsum(ar, r, tag):
        """n2sum[s,co,tp] = sum_i r_i * ar_app[s,i,co,tp] (r=None -> r=1)."""
        av = ar.rearrange("p (s i pa) co tp -> p s i pa co tp",
                          i=4, pa=2)[:, :, :, 1]
        if r is not None:
            w8 = mpool.tile([128, 9, T0, 4, 2], BF16, name=f"w8{tag}",
                            tag="ars_w")[:, :S]
            rv = r.rearrange("p s i (co tp) -> p s i co tp", tp=2)
            nc.vector.tensor_tensor(w8, av, rv, op=AL.mult)
            av = w8
        u1 = mpool.tile([128, 9, 2, 4, 2], F32, name=f"u1{tag}",
                        tag="ars_u")[:, :S]
        nc.vector.tensor_tensor(u1[:, :, 0], av[:, :, 0], av[:, :, 1],
                                op=AL.add)
        nc.vector.tensor_tensor(u1[:, :, 1], av[:, :, 2], av[:, :, 3],
                                op=AL.add)
        n2 = mpool.tile([128, 9, 4, 2], F32, name=f"n2{tag}",
                        tag="st_n2")[:, :S]
        nc.vector.tensor_tensor(n2, u1[:, :, 0], u1[:, :, 1], op=AL.add)
        return n2

    def mult_w_by_p():
        pb = pf.unsqueeze(2).broadcast_to([128, S, T0, 256])
        co_tt(w, Uf, pb, AL.mult, 1)

    def mult_w_by_r(r):
        # r [128, S, T0, 8(co,tp)] -> broadcast over (pa, ab)
        rv = r.rearrange("p s i (co tp) -> p (s i) co tp", tp=2)
        rb = rv.unsqueeze(2).unsqueeze(4).broadcast_to(
            [128, S * 4, 2, 4, 16, 2])
        ub = Uf.rearrange("p s i (pa co ab tp) -> p (s i) pa co ab tp",
                          pa=2, co=4, tp=2)
        wv = wpa.rearrange("p si pa (co ab tp) -> p si pa co ab tp",
                           co=4, tp=2)
        for pa in range(2):
            co_tt(wv[:, :, pa], ub[:, :, pa], rb[:, :, pa], AL.mult, 1)

    def sum_w_into_p():
        co_tt(ts1, w[:, :, 0], w[:, :, 1], AL.add, 1)
        co_tt(ts2, w[:, :, 2], w[:, :, 3], AL.add, 1)
        co_tt(pf, ts1, ts2, AL.add, 1)

    def routstep(ar, sfp, sfa, bacc, first, tag):
        arv = ar.rearrange("p (s i pa) co tp -> p s i pa (co tp)",
                           i=4, pa=2)
        ta = mpool.tile([128, 9, T0, 8], BF16, name=f"ta{tag}",
                        tag="rt_ta")[:, :S]
        tb = mpool.tile([128, 9, T0, 8], BF16, name=f"tb{tag}",
                        tag="rt_tb")[:, :S]
        sfpb = sfp.unsqueeze(2).broadcast_to([128, S, T0, 8])
        sfab = sfa.unsqueeze(2).broadcast_to([128, S, T0, 8])
        nc.vector.tensor_tensor(ta, arv[:, :, :, 0], sfpb, op=AL.mult)
        nc.vector.tensor_tensor(tb, arv[:, :, :, 1], sfab, op=AL.mult)
        if first:
            nc.vector.tensor_tensor(bacc, ta, tb, op=AL.mult)
        else:
            nc.vector.tensor_tensor(ta, ta, tb, op=AL.mult)
            nc.vector.tensor_tensor(bacc, bacc, ta, op=AL.add)

    # ---- iter 1 (r = 0.5 folded into scalings) -------------------------
    sfp1 = statP("1")
    mult_w_by_p()
    ar1 = araw("r1")
    sfa1 = statA(arsum(ar1, None, "1"), 0.25, 0.5, "1")
    bacc = rpool.tile([128, 9, T0, 8], F32, name="bacc", tag="bacc",
                      bufs=2)[:, :S]
    routstep(ar1, sfp1, sfa1, bacc, True, "r1")

    # ---- iter 2 --------------------------------------------------------
    r2 = sigmoid_dve(bacc, "r2")
    mult_w_by_r(r2)
    sum_w_into_p()
    sfp2 = statP("2")
    mult_w_by_p()
    ar2 = araw("r2")
    sfa2 = statA(arsum(ar2, r2, "2"), 1.0, 1.0, "2")
    routstep(ar2, sfp2, sfa2, bacc, False, "r2")

    # ---- final ---------------------------------------------------------
    cR = sigmoid_dve(bacc, "cR")
    mult_w_by_r(cR)
    sum_w_into_p()
    sfp3 = statP("3")
    sq = mpool.tile([128, 9, 128], BF16, name="sq3", tag="st_sq", bufs=1)[:, :S]
    nc.vector.tensor_tensor(sq, p[:, :, 1], p[:, :, 1], op=AL.mult)
    sqv = sq.rearrange("p s (co a b tp) -> p s co tp (a b)", co=4, a=4, b=4)
    n2r = mpool.tile([128, 9, 4, 2], F32, name="n2r", tag="st_n2")[:, :S]
    nc.vector.tensor_reduce(n2r, sqv, axis=AX.X, op=AL.add)
    sfa3 = statA(n2r, 1.0, 1.0, "3")
    v3 = rpool.tile([128, 9, 2, 128], BF16, name="v3", tag="v3", bufs=1)
    vv = v3[:, :S].rearrange("p s pa (co ab tp) -> p s pa co ab tp",
                             co=4, tp=2)
    pv = p.rearrange("p s pa (co ab tp) -> p s pa co ab tp", co=4, tp=2)
    sfp3b = sfp3.rearrange("p s (co tp) -> p s co tp", tp=2) \
        .unsqueeze(3).broadcast_to([128, S, 4, 16, 2])
    sfa3b = sfa3.rearrange("p s (co tp) -> p s co tp", tp=2) \
        .unsqueeze(3).broadcast_to([128, S, 4, 16, 2])
    nc.vector.tensor_tensor(vv[:, :, 0], pv[:, :, 0], sfp3b, op=AL.mult)
    nc.vector.tensor_tensor(vv[:, :, 1], pv[:, :, 1], sfa3b, op=AL.mult)
    return v3


def _build_nc():
    nc = bacc.Bacc(None)
    P_d = nc.dram_tensor("patches", [96, T0, NBLK, 4, Z], BF16, kind="ExternalInput")
    CW_d = nc.dram_tensor("convw", [96, T0, 32, 8], BF16, kind="ExternalInput")
    TW_d = nc.dram_tensor("tw", [128, T0, 2, 128], BF16, kind="ExternalInput")
    RW_d = nc.dram_tensor("rw", [128, 4, 32], BF16, kind="ExternalInput")
    KA_d = nc.dram_tensor("ka", [128, T0], F32, kind="ExternalInput")
    XY_d = nc.dram_tensor("xy", [128, NBLK, 2, 32], BF16, kind="ExternalInput")
    ID_d = nc.dram_tensor("ident", [128, 128], BF16, kind="ExternalInput")
    OUT_d = nc.dram_tensor("out", [256, NBLK, 128], BF16, kind="ExternalOutput")

    with tile.TileContext(nc) as tc:
        with (
            tc.tile_pool(name="const", bufs=1) as cpool,
            tc.tile_pool(name="pload", bufs=2) as ppool,
            tc.tile_pool(name="xbuf", bufs=2) as xpool,
            tc.tile_pool(name="stage", bufs=2) as spool,
            tc.tile_pool(name="ubig", bufs=1) as upool,
            tc.tile_pool(name="rscr", bufs=1) as rpool,
            tc.tile_pool(name="small", bufs=2) as mpool,
            tc.tile_pool(name="ps_cv", bufs=2, space="PSUM") as psx,
            tc.tile_pool(name="ps_uh", bufs=3, space="PSUM") as psuh,
            tc.tile_pool(name="ps_ob", bufs=1, space="PSUM") as pso,
        ):
            cw = cpool.tile([96, T0, 32, 8], BF16, name="cw")
            nc.sync.dma_start(cw[:], CW_d[:])
            tw = cpool.tile([128, T0, 2, 128], BF16, name="tw")
            nc.sync.dma_start(tw[:], TW_d[:])
            rw3 = cpool.tile([128, 4, 32], BF16, name="rw3")
            nc.sync.dma_start(rw3[:], RW_d[:])
            ka = cpool.tile([128, T0], F32, name="ka")
            nc.sync.dma_start(ka[:], KA_d[:])
            xy = cpool.tile([128, NBLK, 2, 32], BF16, name="xy")
            nc.sync.dma_start(xy[:], XY_d[:])
            ident = cpool.tile([128, 128], BF16, name="ident")
            nc.sync.dma_start(ident[:], ID_d[:])

            ubig = upool.tile([128, NBLK, T0, 2, 128], BF16, name="ubig")

            for s0, S in CHUNKS:
                rawt = spool.tile([128, 9, T0, 32], BF16, name="rawt",
                                  tag="rawt", bufs=2)
                xsbs = {}

                def conv_stage(i, s0=s0, S=S):
                    pt = ppool.tile([96, 9, 4, Z], BF16, name="pt", tag="pt")
                    nc.sync.dma_start(pt[:, :S], P_d[:, i, s0:s0 + S])
                    # conv: S data-stationary matmuls, 3-row-batched drains
                    xsb = xpool.tile([128, 9, 32, 8], BF16, name="xsb", tag="xsb")
                    for sg in range(S // 3):
                        cps = psx.tile([128, 3, 256], F32, name="cps", tag="cps")
                        for k in range(3):
                            nc.tensor.matmul(
                                cps[:, k],
                                pt[:, 3 * sg + k].rearrange("p j z -> p (j z)"),
                                cw[:, i].rearrange("p f c -> p (f c)"),
                                start=True, stop=True)
                        nc.scalar.copy(
                            xsb[:, 3 * sg:3 * sg + 3].rearrange(
                                "p s f c -> p (s f c)"),
                            cps[:].rearrange("p s f -> p (s f)"))
                    xsbs[i] = xsb

                def transform_stage(i, s0=s0, S=S):
                    xsb = xsbs[i]
                    # transform pos/app: per (pa, rp) psum [128, S, 32]
                    stp = spool.tile([128, 9, 4, 32], BF16, name="stp", tag="stp")
                    sta = spool.tile([128, 9, 4, 32], BF16, name="sta", tag="sta")
                    for pa, stg_t in ((0, stp), (1, sta)):
                        for rp in range(4):
                            ups = psuh.tile([128, 9, 32], F32, name="ups", tag="uh")
                            for cp in range(4):
                                nc.tensor.matmul(
                                    ups[32 * cp:32 * cp + 32, :S],
                                    tw[32 * rp:32 * rp + 32, i, pa,
                                       32 * cp:32 * cp + 32],
                                    xsb[32 * rp:32 * rp + 32, :S, :, pa * 4 + cp],
                                    start=True, stop=True,
                                    tile_position=(32 * rp, 32 * cp))
                            if pa == 0:
                                nc.scalar.copy(stg_t[:, :S, rp, :], ups[:, :S])
                            else:
                                nc.scalar.activation(
                                    stg_t[:, :S, rp, :], ups[:, :S],
                                    AF.Identity, bias=ka[:, i:i + 1])
                    # raw extraction (pos conv channels, c=3 picks), m''=(co,a,tp)
                    rstg = spool.tile([32, 9, 4, 32], BF16, name="rstg", tag="rstg")
                    for rp in range(4):
                        rfull = psuh.tile([128, 9, 32], F32, name="rps", tag="uh")
                        rps = rfull[0:32, :S]
                        for cp in range(4):
                            nc.tensor.matmul(
                                rps,
                                rw3[32 * rp:32 * rp + 32, cp, :],
                                xsb[32 * rp:32 * rp + 32, :S, :, cp],
                                start=(cp == 0), stop=(cp == 3),
                                tile_position=(32 * rp, 0))
                        nc.scalar.copy(rstg[:, :S, rp, :], rps)
                    # batched DMA-transposes to pixel-major layouts
                    nc.sync.dma_start(
                        ubig[:, s0:s0 + S, i, 0, :],
                        stp[:, :S].rearrange("p s j f -> p (s j f)"),
                        transpose=True)
                    nc.sync.dma_start(
                        ubig[:, s0:s0 + S, i, 1, :],
                        sta[:, :S].rearrange("p s j f -> p (s j f)"),
                        transpose=True)
                    nc.sync.dma_start(
                        rawt[:, :S, i, :],
                        rstg[:, :S].rearrange("p s j f -> p (s j f)"),
                        transpose=True)

                # software pipeline: conv(i+1) is emitted before transform(i)
                # so the PE has runway while ACT drains conv psum
                conv_stage(0)
                for i in range(T0):
                    if i + 1 < T0:
                        conv_stage(i + 1)
                    transform_stage(i)

                # routing for this chunk
                U = ubig[:, s0:s0 + S]
                v3 = _routing_chunk(nc, mpool, rpool, U, rawt[:, :S], xy, s0, S)

                # output: PE transpose + batched drains + one DMA per chunk
                osb = mpool.tile([128, 2, 9, 128], BF16, name="osb", tag="osb",
                                 bufs=1)
                for g in range(S // 3):
                    ops = pso.tile([128, 6, 128], BF16, name="ops", tag="ob")
                    for k in range(6):
                        sp = 6 * g + k
                        s, pa = sp // 2, sp % 2
                        nc.tensor.transpose(ops[:, k], v3[:, s, pa], ident[:])
                    nc.scalar.copy(
                        osb[:, :, 3 * g:3 * g + 3, :],
                        ops[:].rearrange("p (s pa) f -> p pa s f", pa=2))
                nc.sync.dma_start(
                    OUT_d.rearrange("(pa c) b f -> c pa b f", pa=2)[:, :, s0:s0 + S],
                    osb[:, :, :S])
    nc.finalize()
    return nc


_NC_CACHE = None


def _get_nc():
    global _NC_CACHE
    if _NC_CACHE is None:
        _NC_CACHE = _build_nc()
    return _NC_CACHE


def kernel(input_tensor, W_conv, W_pos, W_app, b_app):
    input_tensor = np.asarray(input_tensor, np.float32)
    CW, TW, RW3, KA = _build_weights(np.asarray(W_conv, np.float32),
                                     np.asarray(W_pos, np.float32),
                                     np.asarray(W_app, np.float32),
                                     np.asarray(b_app, np.float32))
    N = input_tensor.shape[0]
    full_pad = np.pad(input_tensor, ((0, 0), (0, 0), (0, 0), (2, 2), (2, 2)))
    bf = np.float16
    ident = np.eye(128, dtype=np.float32)
    in_maps = []
    for c in range(8):
        n, hh = c // 2, c % 2
        sl = full_pad[n, :, :, 48 * hh:48 * hh + 52, :]
        in_maps.append({
            "patches": _build_patches(sl).astype(bf),
            "convw": CW.astype(bf),
            "tw": TW.astype(bf),
            "rw": RW3.astype(bf),
            "ka": KA.astype(np.float32),
            "xy": _pixel_coords(hh).astype(bf),
            "ident": ident.astype(bf),
        })
    nc = _get_nc()
    kres = run_bass_kernel_spmd(nc, in_maps, core_ids=list(range(8)))
    global LAST_RESULT
    LAST_RESULT = kres
    res = kres.results
    # unscramble: out dram [256=(pa,co,a,b,tp), blk36, px128=(j,pi,pj)]
    blk = np.arange(NBLK)
    j = np.arange(4)
    pi = np.arange(4)
    pj = np.arange(8)
    hmap = (4 * (blk // 3))[:, None, None, None] + pi[None, None, :, None]
    hmap = np.broadcast_to(hmap, (NBLK, 4, 4, 8)).ravel()
    wmap = (32 * (blk % 3))[:, None, None, None] + 8 * j[None, :, None, None] \
        + pj[None, None, None, :]
    wmap = np.broadcast_to(wmap, (NBLK, 4, 4, 8)).ravel()
    out = np.zeros((N, T1, Z, H, W), np.float32)
    for c in range(8):
        n, hh = c // 2, c % 2
        img = np.zeros((256, HC, W), np.float32)
        img[:, hmap, wmap] = res[c]["out"].astype(np.float32).reshape(256, NBLK * 128)
        o = img.reshape(2, 4, 4, 4, 2, HC, W)   # [pa, co, a, b, tp]
        for pa in range(2):
            for co in range(4):
                for tp in range(2):
                    t1 = 2 * co + tp
                    zblk = o[pa, co, :, :, tp].reshape(16, HC, W)
                    out[n, t1, pa * 16:pa * 16 + 16,
                        48 * hh:48 * hh + 48] = zblk
    return out


# revision 25
# speedup vs baseline: 1.0206x; 1.0206x over previous
"""Trainium2 Bass kernel for nn_Caps2dMatwo (capsule conv + matwo dual routing).

Sharding: 8 cores = (batch n: 4) x (h-half: 2); each core computes a 48-row
slab of one batch element independently (halo via host padding, no collectives).

v4: u_hat column order m' = (co', a, b, tp) - channel-pure 32-blocks keep the
tile-position transform matmuls valid while making tp (thus t = 2co'+tp)
innermost, so every routing broadcast runs the DVE in 2x mode. Uneven chunks
(3,9,9,9,6 blocks) shrink the pipeline fill/tail. Sigmoid (clamped Pade tanh)
and rsqrt (Quake seed + Newton) run on the DVE so the ACT engine never swaps
function tables and the routing chain never round-trips through ACT. The
app-squash n2 comes from ar (n2 = sum_i r_i*ar_app,i), saving a square+reduce
on iters 1-2. App bias rides the ACT drain (Identity + per-partition bias).
"""
import sys
import numpy as np

sys.path.insert(0, "/opt/trn_rl_repo")

import concourse.bass as bass
import concourse.bacc as bacc
import concourse.mybir as mybir
from concourse import tile
from concourse.bass_utils import run_bass_kernel_spmd
import ml_dtypes

BF16 = mybir.dt.float16
F32 = mybir.dt.float32
AL = mybir.AluOpType
AF = mybir.ActivationFunctionType
AX = mybir.AxisListType

T0, T1, Z, H, W, HC = 4, 8, 32, 96, 96, 48
NBLK = 36
CHUNKS = [(0, 9), (9, 9), (18, 9), (27, 9)]


# ----------------------------------------------------------------------------
# host-side weight/layout construction
# ----------------------------------------------------------------------------

def _build_weights(W_conv, W_pos, W_app, b_app):
    CW = np.zeros((96, T0, 32, 8), np.float32)
    for hi in range(8):
        for wi in range(12):
            for pi in range(4):
                for pj in range(8):
                    dy, dx = hi - pi, wi - pj
                    if 0 <= dy < 5 and 0 <= dx < 5:
                        CW[hi * 12 + wi, :, pi * 8 + pj, :] = W_conv[:, dy, dx, 0, :]

    m_pos = np.stack([W_pos[i].reshape(T1, 4, 4) for i in range(T0)])
    m_app = np.stack([W_app[i].reshape(T1, 4, 4) for i in range(T0)])
    nrm = np.sqrt(np.maximum((m_pos ** 2).sum(axis=2, keepdims=True), 1e-12))
    m_pos = m_pos / nrm

    # m' = co*32 + a*8 + b*2 + tp ; contraction row z = 16tp + 4a + c
    TW = np.zeros((128, T0, 2, 128), np.float32)
    for i in range(T0):
        blkp = np.zeros((32, 128), np.float32)
        blka = np.zeros((32, 128), np.float32)
        for co in range(4):
            for tp in range(2):
                t = 2 * co + tp
                for a in range(4):
                    for b in range(4):
                        m = co * 32 + a * 8 + b * 2 + tp
                        for c in range(4):
                            z = 16 * tp + 4 * a + c
                            blkp[z, m] = m_pos[i, t, c, b]
                            blka[z, m] = m_app[i, t, c, b]
        for j in range(4):
            TW[32 * j:32 * j + 32, i, 0] = blkp
            TW[32 * j:32 * j + 32, i, 1] = blka

    # raw extract: m'' = co*8 + a*2 + tp, one selector block per channel cp
    RW3 = np.zeros((128, 4, 32), np.float32)
    for cp in range(4):
        for tp in range(2):
            for a in range(4):
                z = 16 * tp + 4 * a + 3
                for j in range(4):
                    RW3[32 * j + z, cp, cp * 8 + a * 2 + tp] = 1.0

    KA = np.zeros((128, T0), np.float32)
    for i in range(T0):
        for co in range(4):
            for tp in range(2):
                t = 2 * co + tp
                for a in range(4):
                    for b in range(4):
                        m = co * 32 + a * 8 + b * 2 + tp
                        KA[m, i] = b_app[i, t] * m_app[i, t, :, b].sum()
    return CW, TW, RW3, KA


_PH = np.arange(NBLK) // 3
_B3 = np.arange(NBLK) % 3
_HIDX = (4 * _PH)[:, None] + np.arange(8)[None, :]
_PWJ = (4 * _B3)[:, None] + np.arange(4)[None, :]
_WIDX = (8 * _PWJ)[:, :, None] + np.arange(12)[None, None, :]


def _build_patches(pad):
    g = pad[:, :, _HIDX[:, None, :, None], _WIDX[:, :, None, :]]
    return np.ascontiguousarray(
        g.transpose(4, 5, 0, 2, 3, 1).reshape(96, T0, NBLK, 4, Z))


def _pixel_coords(hh):
    xs = np.zeros((128, NBLK, 2), np.float32)
    for b in range(NBLK):
        ph, b3 = b // 3, b % 3
        for j in range(4):
            for pi in range(4):
                for pj in range(8):
                    part = j * 32 + pi * 8 + pj
                    xs[part, b, 0] = (8 * (4 * b3 + j) + pj) / W
                    xs[part, b, 1] = (4 * ph + pi + 48 * hh) / H
    # pre-broadcast over m''=(co,a,tp) so the coord-add multiply is packed bf16
    return np.ascontiguousarray(
        np.broadcast_to(xs[:, :, :, None], (128, NBLK, 2, 32)))


# ----------------------------------------------------------------------------
# device kernel
# ----------------------------------------------------------------------------

def _routing_chunk(nc, mpool, rpool, U, rawt, xy, s0, S):
    """U: ubig chunk view [128, S, T0, 2, 128(m'=co,a,b,tp)];
    rawt [128, S, T0, 32(m''=co,a,tp)]; xy [128, NBLK, 2, 32] bf16."""
    Uf = U.rearrange("p s i pa c -> p s i (pa c)")        # [128, S, 4, 256]

    # ---- coordinate addition -------------------------------------------
    # U[..., pa=0, co, a, b=k, tp] += xy_k * raw[co, a, tp]
    def co_tt(out, in0, in1, op, sdim):
        nc.vector.tensor_tensor(out, in0, in1, op=op)

    tmpc = mpool.tile([128, 9, T0, 32], BF16, name="tmpc", tag="tmpc", bufs=1)[:, :S]
    Ub = U.rearrange("p s i pa (ca b tp) -> p (s i) pa ca b tp", b=4, tp=2)
    tcb = tmpc.rearrange("p s i (ca tp) -> p (s i) ca tp", tp=2)
    for k in range(2):
        xyb = xy[:, s0:s0 + S, k].unsqueeze(2).broadcast_to([128, S, T0, 32])
        co_tt(tmpc, rawt, xyb, AL.mult, 1)
        usl = Ub[:, :, 0, :, k, :]
        co_tt(usl, tcb, usl, AL.add, 1)

    # ---- p = sum_i U_i (unscaled; r=0.5 folded into stats scalings) ----
    p = rpool.tile([128, 9, 2, 128], BF16, name="p", tag="p", bufs=2)[:, :S]
    ts1 = rpool.tile([128, 9, 256], BF16, name="ts1", tag="ts", bufs=2)[:, :S]
    ts2 = rpool.tile([128, 9, 256], BF16, name="ts2", tag="ts", bufs=2)[:, :S]
    pf = p.rearrange("p s pa c -> p s (pa c)")
    co_tt(ts1, Uf[:, :, 0], Uf[:, :, 1], AL.add, 1)
    co_tt(ts2, Uf[:, :, 2], Uf[:, :, 3], AL.add, 1)
    co_tt(pf, ts1, ts2, AL.add, 1)

    def statP(tag):
        # psquash scale sfp = 1/max_z|p_pos| per t=(co,tp)
        ppos = p[:, :, 0].rearrange("p s (co a b tp) -> p s co tp (a b)",
                                    co=4, a=4, b=4)
        m = mpool.tile([128, 9, 4, 2], F32, name=f"m{tag}", tag="st_m")
        nc.vector.tensor_reduce(m[:, :S], ppos, axis=AX.X, op=AL.max,
                                apply_absolute_value=True)
        sfpf = mpool.tile([128, 9, 8], F32, name=f"sfpf{tag}", tag="st_sfpf")
        nc.vector.reciprocal_approx_fast(
            sfpf[:, :S].rearrange("p s c -> p (s c)"),
            m[:, :S].rearrange("p s c t -> p (s c t)"))
        sfp = mpool.tile([128, 9, 8], BF16, name=f"sfp{tag}", tag=f"sfp{tag}",
                         bufs=1)
        nc.scalar.copy(sfp[:, :S], sfpf[:, :S])
        return sfp[:, :S]

    def statA(n2sum, scale_n2, scale_a, tag):
        # sfa = n2*rsqrt(n2+eps)*scale_a/(1+n2), n2 = scale_n2*n2sum;
        # rsqrt via Quake seed + one Newton step (all on the DVE).
        n2f = n2sum.rearrange("p s c t -> p (s c t)")
        nsq = mpool.tile([128, 9, 8], F32, name=f"nsq{tag}", tag="st_nsq")
        u = nsq[:, :S].rearrange("p s c -> p (s c)")
        nc.vector.tensor_scalar(u, n2f, scale_n2, 1e-9, op0=AL.mult, op1=AL.add)
        y0t = mpool.tile([128, 9, 8], F32, name=f"y0{tag}", tag="st_y0")
        y0 = y0t[:, :S].rearrange("p s c -> p (s c)")
        nc.vector.tensor_scalar(y0.bitcast(mybir.dt.int32),
                                u.bitcast(mybir.dt.int32), 1, None,
                                op0=AL.logical_shift_right)
        # 0x5f3759df - y == (y ^ -1) + 0x5f3759e0 (two's complement)
        nc.vector.tensor_scalar(y0.bitcast(mybir.dt.int32),
                                y0.bitcast(mybir.dt.int32), -1, None,
                                op0=AL.bitwise_xor)
        nc.vector.tensor_scalar(y0.bitcast(mybir.dt.int32),
                                y0.bitcast(mybir.dt.int32), 0x5f3759e0, None,
                                op0=AL.add)
        ht = mpool.tile([128, 9, 8], F32, name=f"h{tag}", tag="st_h")
        h = ht[:, :S].rearrange("p s c -> p (s c)")
        nc.vector.tensor_tensor(h, y0, y0, op=AL.mult)
        nc.vector.tensor_tensor(h, h, u, op=AL.mult)
        nc.vector.tensor_scalar(h, h, -0.5, 1.5, op0=AL.mult, op1=AL.add)
        nc.vector.tensor_tensor(y0, y0, h, op=AL.mult)   # y0 = rsqrt(u)
        den = mpool.tile([128, 9, 8], F32, name=f"den{tag}", tag="st_den")
        dnf = den[:, :S].rearrange("p s c -> p (s c)")
        nc.vector.tensor_scalar(dnf, n2f, scale_n2 / scale_a, 1.0 / scale_a,
                                op0=AL.mult, op1=AL.add)
        rec = mpool.tile([128, 9, 8], F32, name=f"rec{tag}", tag="st_rec")
        rcf = rec[:, :S].rearrange("p s c -> p (s c)")
        nc.vector.reciprocal_approx_fast(rcf, dnf)
        nc.vector.tensor_tensor(y0, y0, rcf, op=AL.mult)
        sfa = mpool.tile([128, 9, 8], BF16, name=f"sfa{tag}", tag=f"sfa{tag}",
                         bufs=1)
        nc.vector.scalar_tensor_tensor(
            sfa[:, :S].rearrange("p s c -> p (s c)"), n2f, scale_n2, y0,
            op0=AL.mult, op1=AL.mult)
        return sfa[:, :S]

    def sigmoid_dve(bacc, name):
        # sigmoid(b) ~= 0.5 + 0.5*pade_tanh(clamp(b/2, +-3)); |err| < 0.005
        bf = bacc.rearrange("p s i c -> p (s i c)")
        xt = mpool.tile([128, 9, T0, 8], F32, name=f"x{name}", tag="sg_x", bufs=1)
        x = xt[:, :S].rearrange("p s i c -> p (s i c)")
        nc.vector.tensor_scalar(x, bf, 0.5, None, op0=AL.mult)
        nc.vector.tensor_scalar(x, x, -3.0, 3.0, op0=AL.max, op1=AL.min)
        dt_ = mpool.tile([128, 9, T0, 8], F32, name=f"d{name}", tag="sg_d", bufs=1)
        dd = dt_[:, :S].rearrange("p s i c -> p (s i c)")
        nc.vector.tensor_tensor(dd, x, x, op=AL.mult)     # x^2
        tt = mpool.tile([128, 9, T0, 8], F32, name=f"t{name}", tag="sg_t", bufs=1)
        t = tt[:, :S].rearrange("p s i c -> p (s i c)")
        nc.vector.scalar_tensor_tensor(t, dd, 27.0, x, op0=AL.add, op1=AL.mult)
        nc.vector.tensor_scalar(dd, dd, 9.0, 27.0, op0=AL.mult, op1=AL.add)
        nc.vector.reciprocal_approx_fast(dd, dd)
        nc.vector.tensor_tensor(t, t, dd, op=AL.mult)
        r = rpool.tile([128, 9, T0, 8], BF16, name=name, tag="r2", bufs=2)
        nc.vector.tensor_scalar(r[:, :S].rearrange("p s i c -> p (s i c)"),
                                t, 0.5, 0.5, op0=AL.mult, op1=AL.add)
        return r[:, :S]

    w = rpool.tile([128, 9, T0, 256], BF16, name="w", tag="w", bufs=2)[:, :S]
    wpa = w.rearrange("p s i (pa co ab tp) -> p (s i) pa (co ab tp)",
                      pa=2, co=4, tp=2)

    def araw(tag):
        """w holds U*p'; reduce z=(a,b) -> ar [128, 8S(sipa), 4co, 2tp].
        b-level-1 is in place in w (w is consumed; next mult rewrites it)."""
        wz = w.rearrange("p s i (pa ca b tp) -> p (s i) pa ca b tp",
                         pa=2, b=4, tp=2)
        for pa in range(2):          # b: 4 -> 2, in place into b0:2
            co_tt(wz[:, :, pa, :, 0:2], wz[:, :, pa, :, 0:2],
                  wz[:, :, pa, :, 2:4], AL.add, 1)
        t2 = rpool.tile([128, 36, 2, 16, 2], BF16, name=f"t2{tag}",
                        tag="t2", bufs=1)[:, :4 * S]
        for pa in range(2):          # b: 2 -> 1
            co_tt(t2[:, :, pa], wz[:, :, pa, :, 0], wz[:, :, pa, :, 1],
                  AL.add, 1)
        t2v = t2.rearrange("p si pa (co a) tp -> p (si pa) co a tp", a=4)
        t3 = rpool.tile([128, 72, 4, 2, 2], BF16, name=f"t3{tag}",
                        tag="t3", bufs=1)[:, :8 * S]
        nc.vector.tensor_tensor(t3, t2v[:, :, :, 0:2], t2v[:, :, :, 2:4],
                                op=AL.add)
        ar = rpool.tile([128, 72, 4, 2], BF16, name=f"ar{tag}", tag="ar",
                        bufs=2)[:, :8 * S]
        nc.vector.tensor_tensor(ar, t3[:, :, :, 0], t3[:, :, :, 1],
                                op=AL.add)
        return ar

    def arsum(ar, r, tag):
        """n2sum[s,co,tp] = sum_i r_i * ar_app[s,i,co,tp] (r=None -> r=1)."""
        av = ar.rearrange("p (s i pa) co tp -> p s i pa co tp",
                          i=4, pa=2)[:, :, :, 1]
        if r is not None:
            w8 = mpool.tile([128, 9, T0, 4, 2], BF16, name=f"w8{tag}",
                            tag="ars_w")[:, :S]
            rv = r.rearrange("p s i (co tp) -> p s i co tp", tp=2)
            nc.vector.tensor_tensor(w8, av, rv, op=AL.mult)
            av = w8
        u1 = mpool.tile([128, 9, 2, 4, 2], F32, name=f"u1{tag}",
                        tag="ars_u")[:, :S]
        nc.vector.tensor_tensor(u1[:, :, 0], av[:, :, 0], av[:, :, 1],
                                op=AL.add)
        nc.vector.tensor_tensor(u1[:, :, 1], av[:, :, 2], av[:, :, 3],
                                op=AL.add)
        n2 = mpool.tile([128, 9, 4, 2], F32, name=f"n2{tag}",
                        tag="st_n2")[:, :S]
        nc.vector.tensor_tensor(n2, u1[:, :, 0], u1[:, :, 1], op=AL.add)
        return n2

    def mult_w_by_p():
        pb = pf.unsqueeze(2).broadcast_to([128, S, T0, 256])
        co_tt(w, Uf, pb, AL.mult, 1)

    def mult_w_by_r(r):
        # r [128, S, T0, 8(co,tp)] -> broadcast over (pa, ab)
        rv = r.rearrange("p s i (co tp) -> p (s i) co tp", tp=2)
        rb = rv.unsqueeze(2).unsqueeze(4).broadcast_to(
            [128, S * 4, 2, 4, 16, 2])
        ub = Uf.rearrange("p s i (pa co ab tp) -> p (s i) pa co ab tp",
                          pa=2, co=4, tp=2)
        wv = wpa.rearrange("p si pa (co ab tp) -> p si pa co ab tp",
                           co=4, tp=2)
        for pa in range(2):
            co_tt(wv[:, :, pa], ub[:, :, pa], rb[:, :, pa], AL.mult, 1)

    def sum_w_into_p():
        co_tt(ts1, w[:, :, 0], w[:, :, 1], AL.add, 1)
        co_tt(ts2, w[:, :, 2], w[:, :, 3], AL.add, 1)
        co_tt(pf, ts1, ts2, AL.add, 1)

    def routstep(ar, sfp, sfa, bacc, first, tag):
        arv = ar.rearrange("p (s i pa) co tp -> p s i pa (co tp)",
                           i=4, pa=2)
        ta = mpool.tile([128, 9, T0, 8], BF16, name=f"ta{tag}",
                        tag="rt_ta")[:, :S]
        tb = mpool.tile([128, 9, T0, 8], BF16, name=f"tb{tag}",
                        tag="rt_tb")[:, :S]
        sfpb = sfp.unsqueeze(2).broadcast_to([128, S, T0, 8])
        sfab = sfa.unsqueeze(2).broadcast_to([128, S, T0, 8])
        nc.vector.tensor_tensor(ta, arv[:, :, :, 0], sfpb, op=AL.mult)
        nc.vector.tensor_tensor(tb, arv[:, :, :, 1], sfab, op=AL.mult)
        if first:
            nc.vector.tensor_tensor(bacc, ta, tb, op=AL.mult)
        else:
            nc.vector.tensor_tensor(ta, ta, tb, op=AL.mult)
            nc.vector.tensor_tensor(bacc, bacc, ta, op=AL.add)

    # ---- iter 1 (r = 0.5 folded into scalings) -------------------------
    sfp1 = statP("1")
    mult_w_by_p()
    ar1 = araw("r1")
    sfa1 = statA(arsum(ar1, None, "1"), 0.25, 0.5, "1")
    bacc = rpool.tile([128, 9, T0, 8], F32, name="bacc", tag="bacc",
                      bufs=2)[:, :S]
    routstep(ar1, sfp1, sfa1, bacc, True, "r1")

    # ---- iter 2 --------------------------------------------------------
    r2 = sigmoid_dve(bacc, "r2")
    mult_w_by_r(r2)
    sum_w_into_p()
    sfp2 = statP("2")
    mult_w_by_p()
    ar2 = araw("r2")
    sfa2 = statA(arsum(ar2, r2, "2"), 1.0, 1.0, "2")
    routstep(ar2, sfp2, sfa2, bacc, False, "r2")

    # ---- final ---------------------------------------------------------
    cR = sigmoid_dve(bacc, "cR")
    mult_w_by_r(cR)
    sum_w_into_p()
    sfp3 = statP("3")
    sq = mpool.tile([128, 9, 128], BF16, name="sq3", tag="st_sq", bufs=1)[:, :S]
    nc.vector.tensor_tensor(sq, p[:, :, 1], p[:, :, 1], op=AL.mult)
    sqv = sq.rearrange("p s (co a b tp) -> p s co tp (a b)", co=4, a=4, b=4)
    n2r = mpool.tile([128, 9, 4, 2], F32, name="n2r", tag="st_n2")[:, :S]
    nc.vector.tensor_reduce(n2r, sqv, axis=AX.X, op=AL.add)
    sfa3 = statA(n2r, 1.0, 1.0, "3")
    v3 = rpool.tile([128, 9, 2, 128], BF16, name="v3", tag="v3", bufs=1)
    vv = v3[:, :S].rearrange("p s pa (co ab tp) -> p s pa co ab tp",
                             co=4, tp=2)
    pv = p.rearrange("p s pa (co ab tp) -> p s pa co ab tp", co=4, tp=2)
    sfp3b = sfp3.rearrange("p s (co tp) -> p s co tp", tp=2) \
        .unsqueeze(3).broadcast_to([128, S, 4, 16, 2])
    sfa3b = sfa3.rearrange("p s (co tp) -> p s co tp", tp=2) \
        .unsqueeze(3).broadcast_to([128, S, 4, 16, 2])
    nc.vector.tensor_tensor(vv[:, :, 0], pv[:, :, 0], sfp3b, op=AL.mult)
    nc.vector.tensor_tensor(vv[:, :, 1], pv[:, :, 1], sfa3b, op=AL.mult)
    return v3


def _build_nc():
    nc = bacc.Bacc(None)
    P_d = nc.dram_tensor("patches", [96, T0, NBLK, 4, Z], BF16, kind="ExternalInput")
    CW_d = nc.dram_tensor("convw", [96, T0, 32, 8], BF16, kind="ExternalInput")
    TW_d = nc.dram_tensor("tw", [128, T0, 2, 128], BF16, kind="ExternalInput")
    RW_d = nc.dram_tensor("rw", [128, 4, 32], BF16, kind="ExternalInput")
    KA_d = nc.dram_tensor("ka", [128, T0], F32, kind="ExternalInput")
    XY_d = nc.dram_tensor("xy", [128, NBLK, 2, 32], BF16, kind="ExternalInput")
    ID_d = nc.dram_tensor("ident", [128, 128], BF16, kind="ExternalInput")
    OUT_d = nc.dram_tensor("out", [256, NBLK, 128], BF16, kind="ExternalOutput")

    with tile.TileContext(nc) as tc:
        with (
            tc.tile_pool(name="const", bufs=1) as cpool,
            tc.tile_pool(name="pload", bufs=2) as ppool,
            tc.tile_pool(name="xbuf", bufs=2) as xpool,
            tc.tile_pool(name="stage", bufs=2) as spool,
            tc.tile_pool(name="ubig", bufs=1) as upool,
            tc.tile_pool(name="rscr", bufs=1) as rpool,
            tc.tile_pool(name="small", bufs=2) as mpool,
            tc.tile_pool(name="ps_cv", bufs=2, space="PSUM") as psx,
            tc.tile_pool(name="ps_uh", bufs=3, space="PSUM") as psuh,
            tc.tile_pool(name="ps_ob", bufs=1, space="PSUM") as pso,
        ):
            cw = cpool.tile([96, T0, 32, 8], BF16, name="cw")
            nc.sync.dma_start(cw[:], CW_d[:])
            tw = cpool.tile([128, T0, 2, 128], BF16, name="tw")
            nc.sync.dma_start(tw[:], TW_d[:])
            rw3 = cpool.tile([128, 4, 32], BF16, name="rw3")
            nc.sync.dma_start(rw3[:], RW_d[:])
            ka = cpool.tile([128, T0], F32, name="ka")
            nc.sync.dma_start(ka[:], KA_d[:])
            xy = cpool.tile([128, NBLK, 2, 32], BF16, name="xy")
            nc.sync.dma_start(xy[:], XY_d[:])
            ident = cpool.tile([128, 128], BF16, name="ident")
            nc.sync.dma_start(ident[:], ID_d[:])

            ubig = upool.tile([128, NBLK, T0, 2, 128], BF16, name="ubig")

            for s0, S in CHUNKS:
                rawt = spool.tile([128, 9, T0, 32], BF16, name="rawt",
                                  tag="rawt", bufs=2)
                xsbs = {}

                def conv_stage(i, s0=s0, S=S):
                    pt = ppool.tile([96, 9, 4, Z], BF16, name="pt", tag="pt")
                    nc.sync.dma_start(pt[:, :S], P_d[:, i, s0:s0 + S])
                    # conv: S data-stationary matmuls, 3-row-batched drains
                    xsb = xpool.tile([128, 9, 32, 8], BF16, name="xsb", tag="xsb")
                    for sg in range(S // 3):
                        cps = psx.tile([128, 3, 256], F32, name="cps", tag="cps")
                        for k in range(3):
                            nc.tensor.matmul(
                                cps[:, k],
                                pt[:, 3 * sg + k].rearrange("p j z -> p (j z)"),
                                cw[:, i].rearrange("p f c -> p (f c)"),
                                start=True, stop=True)
                        nc.scalar.copy(
                            xsb[:, 3 * sg:3 * sg + 3].rearrange(
                                "p s f c -> p (s f c)"),
                            cps[:].rearrange("p s f -> p (s f)"))
                    xsbs[i] = xsb

                def transform_stage(i, s0=s0, S=S):
                    xsb = xsbs[i]
                    # transform pos/app: per (pa, rp) psum [128, S, 32]
                    stp = spool.tile([128, 9, 4, 32], BF16, name="stp", tag="stp")
                    sta = spool.tile([128, 9, 4, 32], BF16, name="sta", tag="sta")
                    for pa, stg_t in ((0, stp), (1, sta)):
                        for rp in range(4):
                            ups = psuh.tile([128, 9, 32], F32, name="ups", tag="uh")
                            for cp in range(4):
                                nc.tensor.matmul(
                                    ups[32 * cp:32 * cp + 32, :S],
                                    tw[32 * rp:32 * rp + 32, i, pa,
                                       32 * cp:32 * cp + 32],
                                    xsb[32 * rp:32 * rp + 32, :S, :, pa * 4 + cp],
                                    start=True, stop=True,
                                    tile_position=(32 * rp, 32 * cp))
                            if pa == 0:
                                nc.scalar.copy(stg_t[:, :S, rp, :], ups[:, :S])
                            else:
                                nc.scalar.activation(
                                    stg_t[:, :S, rp, :], ups[:, :S],
                                    AF.Identity, bias=ka[:, i:i + 1])
                    # raw extraction (pos conv channels, c=3 picks), m''=(co,a,tp)
                    rstg = spool.tile([32, 9, 4, 32], BF16, name="rstg", tag="rstg")
                    for rp in range(4):
                        rfull = psuh.tile([128, 9, 32], F32, name="rps", tag="uh")
                        rps = rfull[0:32, :S]
                        for cp in range(4):
                            nc.tensor.matmul(
                                rps,
                                rw3[32 * rp:32 * rp + 32, cp, :],
                                xsb[32 * rp:32 * rp + 32, :S, :, cp],
                                start=(cp == 0), stop=(cp == 3),
                                tile_position=(32 * rp, 0))
                        nc.scalar.copy(rstg[:, :S, rp, :], rps)
                    # batched DMA-transposes to pixel-major layouts
                    nc.sync.dma_start(
                        ubig[:, s0:s0 + S, i, 0, :],
                        stp[:, :S].rearrange("p s j f -> p (s j f)"),
                        transpose=True)
                    nc.sync.dma_start(
                        ubig[:, s0:s0 + S, i, 1, :],
                        sta[:, :S].rearrange("p s j f -> p (s j f)"),
                        transpose=True)
                    nc.sync.dma_start(
                        rawt[:, :S, i, :],
                        rstg[:, :S].rearrange("p s j f -> p (s j f)"),
                        transpose=True)

                # software pipeline: conv(i+1) is emitted before transform(i)
                # so the PE has runway while ACT drains conv psum
                conv_stage(0)
                for i in range(T0):
                    if i + 1 < T0:
                        conv_stage(i + 1)
                    transform_stage(i)

                # routing for this chunk
                U = ubig[:, s0:s0 + S]
                v3 = _routing_chunk(nc, mpool, rpool, U, rawt[:, :S], xy, s0, S)

                # output: PE transpose + batched drains + one DMA per chunk
                osb = mpool.tile([128, 2, 9, 128], BF16, name="osb", tag="osb",
                                 bufs=1)
                for g in range(S // 3):
                    ops = pso.tile([128, 6, 128], BF16, name="ops", tag="ob")
                    for k in range(6):
                        sp = 6 * g + k
                        s, pa = sp // 2, sp % 2
                        nc.tensor.transpose(ops[:, k], v3[:, s, pa], ident[:])
                    nc.scalar.copy(
                        osb[:, :, 3 * g:3 * g + 3, :],
                        ops[:].rearrange("p (s pa) f -> p pa s f", pa=2))
                nc.sync.dma_start(
                    OUT_d.rearrange("(pa c) b f -> c pa b f", pa=2)[:, :, s0:s0 + S],
                    osb[:, :, :S])
    nc.finalize()
    return nc


_NC_CACHE = None


def _get_nc():
    global _NC_CACHE
    if _NC_CACHE is None:
        _NC_CACHE = _build_nc()
    return _NC_CACHE


def kernel(input_tensor, W_conv, W_pos, W_app, b_app):
    input_tensor = np.asarray(input_tensor, np.float32)
    CW, TW, RW3, KA = _build_weights(np.asarray(W_conv, np.float32),
                                     np.asarray(W_pos, np.float32),
                                     np.asarray(W_app, np.float32),
                                     np.asarray(b_app, np.float32))
    N = input_tensor.shape[0]
    full_pad = np.pad(input_tensor, ((0, 0), (0, 0), (0, 0), (2, 2), (2, 2)))
    bf = np.float16
    ident = np.eye(128, dtype=np.float32)
    in_maps = []
    for c in range(8):
        n, hh = c // 2, c % 2
        sl = full_pad[n, :, :, 48 * hh:48 * hh + 52, :]
        in_maps.append({
            "patches": _build_patches(sl).astype(bf),
            "convw": CW.astype(bf),
            "tw": TW.astype(bf),
            "rw": RW3.astype(bf),
            "ka": KA.astype(np.float32),
            "xy": _pixel_coords(hh).astype(bf),
            "ident": ident.astype(bf),
        })
    nc = _get_nc()
    kres = run_bass_kernel_spmd(nc, in_maps, core_ids=list(range(8)))
    global LAST_RESULT
    LAST_RESULT = kres
    res = kres.results
    # unscramble: out dram [256=(pa,co,a,b,tp), blk36, px128=(j,pi,pj)]
    blk = np.arange(NBLK)
    j = np.arange(4)
    pi = np.arange(4)
    pj = np.arange(8)
    hmap = (4 * (blk // 3))[:, None, None, None] + pi[None, None, :, None]
    hmap = np.broadcast_to(hmap, (NBLK, 4, 4, 8)).ravel()
    wmap = (32 * (blk % 3))[:, None, None, None] + 8 * j[None, :, None, None] \
        + pj[None, None, None, :]
    wmap = np.broadcast_to(wmap, (NBLK, 4, 4, 8)).ravel()
    out = np.zeros((N, T1, Z, H, W), np.float32)
    for c in range(8):
        n, hh = c // 2, c % 2
        img = np.zeros((256, HC, W), np.float32)
        img[:, hmap, wmap] = res[c]["out"].astype(np.float32).reshape(256, NBLK * 128)
        o = img.reshape(2, 4, 4, 4, 2, HC, W)   # [pa, co, a, b, tp]
        for pa in range(2):
            for co in range(4):
                for tp in range(2):
                    t1 = 2 * co + tp
                    zblk = o[pa, co, :, :, tp].reshape(16, HC, W)
                    out[n, t1, pa * 16:pa * 16 + 16,
                        48 * hh:48 * hh + 48] = zblk
    return out


# revision 26
# speedup vs baseline: 1.0780x; 1.0563x over previous
"""Trainium2 Bass kernel for nn_Caps2dMatwo (capsule conv + matwo dual routing).

Sharding: 8 cores = (batch n: 4) x (h-half: 2); each core computes a 48-row
slab of one batch element independently (halo via host padding, no collectives).

v4: u_hat column order m' = (co', a, b, tp) - channel-pure 32-blocks keep the
tile-position transform matmuls valid while making tp (thus t = 2co'+tp)
innermost, so every routing broadcast runs the DVE in 2x mode. Uneven chunks
(3,9,9,9,6 blocks) shrink the pipeline fill/tail. Sigmoid (clamped Pade tanh)
and rsqrt (Quake seed + Newton) run on the DVE so the ACT engine never swaps
function tables and the routing chain never round-trips through ACT. The
app-squash n2 comes from ar (n2 = sum_i r_i*ar_app,i), saving a square+reduce
on iters 1-2. App bias rides the ACT drain (Identity + per-partition bias).
"""
import sys
import numpy as np

sys.path.insert(0, "/opt/trn_rl_repo")

import concourse.bass as bass
import concourse.bacc as bacc
import concourse.mybir as mybir
from concourse import tile
from concourse.bass_utils import run_bass_kernel_spmd
import ml_dtypes

BF16 = mybir.dt.float16
F32 = mybir.dt.float32
AL = mybir.AluOpType
AF = mybir.ActivationFunctionType
AX = mybir.AxisListType

T0, T1, Z, H, W, HC = 4, 8, 32, 96, 96, 48
NBLK = 36
CHUNKS = [(0, 9), (9, 9), (18, 9), (27, 9)]


# ----------------------------------------------------------------------------
# host-side weight/layout construction
# ----------------------------------------------------------------------------

def _build_weights(W_conv, W_pos, W_app, b_app):
    CW = np.zeros((96, T0, 32, 8), np.float32)
    for hi in range(8):
        for wi in range(12):
            for pi in range(4):
                for pj in range(8):
                    dy, dx = hi - pi, wi - pj
                    if 0 <= dy < 5 and 0 <= dx < 5:
                        CW[hi * 12 + wi, :, pi * 8 + pj, :] = W_conv[:, dy, dx, 0, :]

    m_pos = np.stack([W_pos[i].reshape(T1, 4, 4) for i in range(T0)])
    m_app = np.stack([W_app[i].reshape(T1, 4, 4) for i in range(T0)])
    nrm = np.sqrt(np.maximum((m_pos ** 2).sum(axis=2, keepdims=True), 1e-12))
    m_pos = m_pos / nrm

    # m' = co*32 + a*8 + b*2 + tp ; contraction row z = 16tp + 4a + c
    TW = np.zeros((128, T0, 2, 128), np.float32)
    for i in range(T0):
        blkp = np.zeros((32, 128), np.float32)
        blka = np.zeros((32, 128), np.float32)
        for co in range(4):
            for tp in range(2):
                t = 2 * co + tp
                for a in range(4):
                    for b in range(4):
                        m = co * 32 + a * 8 + b * 2 + tp
                        for c in range(4):
                            z = 16 * tp + 4 * a + c
                            blkp[z, m] = m_pos[i, t, c, b]
                            blka[z, m] = m_app[i, t, c, b]
        for j in range(4):
            TW[32 * j:32 * j + 32, i, 0] = blkp
            TW[32 * j:32 * j + 32, i, 1] = blka

    # raw extract: m'' = co*8 + a*2 + tp, one selector block per channel cp
    RW3 = np.zeros((128, 4, 32), np.float32)
    for cp in range(4):
        for tp in range(2):
            for a in range(4):
                z = 16 * tp + 4 * a + 3
                for j in range(4):
                    RW3[32 * j + z, cp, cp * 8 + a * 2 + tp] = 1.0

    KA = np.zeros((128, T0), np.float32)
    for i in range(T0):
        for co in range(4):
            for tp in range(2):
                t = 2 * co + tp
                for a in range(4):
                    for b in range(4):
                        m = co * 32 + a * 8 + b * 2 + tp
                        KA[m, i] = b_app[i, t] * m_app[i, t, :, b].sum()
    return CW, TW, RW3, KA


_PH = np.arange(NBLK) // 3
_B3 = np.arange(NBLK) % 3
_HIDX = (4 * _PH)[:, None] + np.arange(8)[None, :]
_PWJ = (4 * _B3)[:, None] + np.arange(4)[None, :]
_WIDX = (8 * _PWJ)[:, :, None] + np.arange(12)[None, None, :]


def _build_patches(pad):
    g = pad[:, :, _HIDX[:, None, :, None], _WIDX[:, :, None, :]]
    return np.ascontiguousarray(
        g.transpose(4, 5, 0, 2, 3, 1).reshape(96, T0, NBLK, 4, Z))


def _pixel_coords(hh):
    xs = np.zeros((128, NBLK, 2), np.float32)
    for b in range(NBLK):
        ph, b3 = b // 3, b % 3
        for j in range(4):
            for pi in range(4):
                for pj in range(8):
                    part = j * 32 + pi * 8 + pj
                    xs[part, b, 0] = (8 * (4 * b3 + j) + pj) / W
                    xs[part, b, 1] = (4 * ph + pi + 48 * hh) / H
    # pre-broadcast over m''=(co,a,tp) so the coord-add multiply is packed bf16
    return np.ascontiguousarray(
        np.broadcast_to(xs[:, :, :, None], (128, NBLK, 2, 32)))


# ----------------------------------------------------------------------------
# device kernel
# ----------------------------------------------------------------------------

def _routing_chunk(nc, mpool, rpool, U, rawt, xy, s0, S):
    """U: ubig chunk view [128, S, T0, 2, 128(m'=co,a,b,tp)];
    rawt [128, S, T0, 32(m''=co,a,tp)]; xy [128, NBLK, 2, 32] bf16."""
    Uf = U.rearrange("p s i pa c -> p s i (pa c)")        # [128, S, 4, 256]

    # ---- coordinate addition -------------------------------------------
    # U[..., pa=0, co, a, b=k, tp] += xy_k * raw[co, a, tp]
    def co_tt(out, in0, in1, op, sdim):
        nc.vector.tensor_tensor(out, in0, in1, op=op)

    tmpc = mpool.tile([128, 9, T0, 32], BF16, name="tmpc", tag="tmpc", bufs=1)[:, :S]
    Ub = U.rearrange("p s i pa (ca b tp) -> p (s i) pa ca b tp", b=4, tp=2)
    tcb = tmpc.rearrange("p s i (ca tp) -> p (s i) ca tp", tp=2)
    for k in range(2):
        xyb = xy[:, s0:s0 + S, k].unsqueeze(2).broadcast_to([128, S, T0, 32])
        co_tt(tmpc, rawt, xyb, AL.mult, 1)
        usl = Ub[:, :, 0, :, k, :]
        co_tt(usl, tcb, usl, AL.add, 1)

    # ---- p = sum_i U_i (unscaled; r=0.5 folded into stats scalings) ----
    p = rpool.tile([128, 9, 2, 128], BF16, name="p", tag="p", bufs=2)[:, :S]
    ts1 = rpool.tile([128, 9, 256], BF16, name="ts1", tag="ts", bufs=2)[:, :S]
    ts2 = rpool.tile([128, 9, 256], BF16, name="ts2", tag="ts", bufs=2)[:, :S]
    pf = p.rearrange("p s pa c -> p s (pa c)")
    co_tt(ts1, Uf[:, :, 0], Uf[:, :, 1], AL.add, 1)
    co_tt(ts2, Uf[:, :, 2], Uf[:, :, 3], AL.add, 1)
    co_tt(pf, ts1, ts2, AL.add, 1)

    def statP(tag):
        # psquash scale sfp = 1/max_z|p_pos| per t=(co,tp)
        ppos = p[:, :, 0].rearrange("p s (co a b tp) -> p s co tp (a b)",
                                    co=4, a=4, b=4)
        m = mpool.tile([128, 9, 4, 2], F32, name=f"m{tag}", tag="st_m")
        nc.vector.tensor_reduce(m[:, :S], ppos, axis=AX.X, op=AL.max,
                                apply_absolute_value=True)
        sfpf = mpool.tile([128, 9, 8], F32, name=f"sfpf{tag}", tag="st_sfpf")
        nc.vector.reciprocal_approx_fast(
            sfpf[:, :S].rearrange("p s c -> p (s c)"),
            m[:, :S].rearrange("p s c t -> p (s c t)"))
        sfp = mpool.tile([128, 9, 8], BF16, name=f"sfp{tag}", tag=f"sfp{tag}",
                         bufs=1)
        nc.scalar.copy(sfp[:, :S], sfpf[:, :S])
        return sfp[:, :S]

    def statA(n2sum, scale_n2, scale_a, tag):
        # sfa = n2*rsqrt(n2+eps)*scale_a/(1+n2), n2 = scale_n2*n2sum;
        # rsqrt via Quake seed + one Newton step (all on the DVE).
        n2f = n2sum.rearrange("p s c t -> p (s c t)")
        nsq = mpool.tile([128, 9, 8], F32, name=f"nsq{tag}", tag="st_nsq")
        u = nsq[:, :S].rearrange("p s c -> p (s c)")
        nc.vector.tensor_scalar(u, n2f, scale_n2, 1e-9, op0=AL.mult, op1=AL.add)
        y0t = mpool.tile([128, 9, 8], F32, name=f"y0{tag}", tag="st_y0")
        y0 = y0t[:, :S].rearrange("p s c -> p (s c)")
        nc.vector.tensor_scalar(y0.bitcast(mybir.dt.int32),
                                u.bitcast(mybir.dt.int32), 1, None,
                                op0=AL.logical_shift_right)
        # 0x5f3759df - y == (y ^ -1) + 0x5f3759e0 (two's complement)
        nc.vector.tensor_scalar(y0.bitcast(mybir.dt.int32),
                                y0.bitcast(mybir.dt.int32), -1, None,
                                op0=AL.bitwise_xor)
        nc.vector.tensor_scalar(y0.bitcast(mybir.dt.int32),
                                y0.bitcast(mybir.dt.int32), 0x5f3759e0, None,
                                op0=AL.add)
        ht = mpool.tile([128, 9, 8], F32, name=f"h{tag}", tag="st_h")
        h = ht[:, :S].rearrange("p s c -> p (s c)")
        nc.vector.tensor_tensor(h, y0, y0, op=AL.mult)
        nc.vector.tensor_tensor(h, h, u, op=AL.mult)
        nc.vector.tensor_scalar(h, h, -0.5, 1.5, op0=AL.mult, op1=AL.add)
        nc.vector.tensor_tensor(y0, y0, h, op=AL.mult)   # y0 = rsqrt(u)
        den = mpool.tile([128, 9, 8], F32, name=f"den{tag}", tag="st_den")
        dnf = den[:, :S].rearrange("p s c -> p (s c)")
        nc.vector.tensor_scalar(dnf, n2f, scale_n2 / scale_a, 1.0 / scale_a,
                                op0=AL.mult, op1=AL.add)
        rec = mpool.tile([128, 9, 8], F32, name=f"rec{tag}", tag="st_rec")
        rcf = rec[:, :S].rearrange("p s c -> p (s c)")
        nc.vector.reciprocal_approx_fast(rcf, dnf)
        nc.vector.tensor_tensor(y0, y0, rcf, op=AL.mult)
        sfa = mpool.tile([128, 9, 8], BF16, name=f"sfa{tag}", tag=f"sfa{tag}",
                         bufs=1)
        nc.vector.scalar_tensor_tensor(
            sfa[:, :S].rearrange("p s c -> p (s c)"), n2f, scale_n2, y0,
            op0=AL.mult, op1=AL.mult)
        return sfa[:, :S]

    def sigmoid_dve(bacc, name):
        # sigmoid(b) ~= 0.5 + 0.5*pade_tanh(clamp(b/2, +-3)); |err| < 0.005
        bf = bacc.rearrange("p s i c -> p (s i c)")
        xt = mpool.tile([128, 9, T0, 8], F32, name=f"x{name}", tag="sg_x", bufs=1)
        x = xt[:, :S].rearrange("p s i c -> p (s i c)")
        nc.vector.tensor_scalar(x, bf, 0.5, None, op0=AL.mult)
        nc.vector.tensor_scalar(x, x, -3.0, 3.0, op0=AL.max, op1=AL.min)
        dt_ = mpool.tile([128, 9, T0, 8], F32, name=f"d{name}", tag="sg_d", bufs=1)
        dd = dt_[:, :S].rearrange("p s i c -> p (s i c)")
        nc.vector.tensor_tensor(dd, x, x, op=AL.mult)     # x^2
        tt = mpool.tile([128, 9, T0, 8], F32, name=f"t{name}", tag="sg_t", bufs=1)
        t = tt[:, :S].rearrange("p s i c -> p (s i c)")
        nc.vector.scalar_tensor_tensor(t, dd, 27.0, x, op0=AL.add, op1=AL.mult)
        nc.vector.tensor_scalar(dd, dd, 9.0, 27.0, op0=AL.mult, op1=AL.add)
        nc.vector.reciprocal_approx_fast(dd, dd)
        nc.vector.tensor_tensor(t, t, dd, op=AL.mult)
        r = rpool.tile([128, 9, T0, 8], BF16, name=name, tag="r2", bufs=2)
        nc.vector.tensor_scalar(r[:, :S].rearrange("p s i c -> p (s i c)"),
                                t, 0.5, 0.5, op0=AL.mult, op1=AL.add)
        return r[:, :S]

    w = rpool.tile([128, 9, T0, 256], BF16, name="w", tag="w", bufs=2)[:, :S]
    wpa = w.rearrange("p s i (pa co ab tp) -> p (s i) pa (co ab tp)",
                      pa=2, co=4, tp=2)

    def araw(tag):
        """w holds U*p'; reduce z=(a,b) -> ar [128, 8S(sipa), 4co, 2tp].
        b-level-1 is in place in w (w is consumed; next mult rewrites it)."""
        wz = w.rearrange("p s i (pa ca b tp) -> p (s i) pa ca b tp",
                         pa=2, b=4, tp=2)
        for pa in range(2):          # b: 4 -> 2, in place into b0:2
            co_tt(wz[:, :, pa, :, 0:2], wz[:, :, pa, :, 0:2],
                  wz[:, :, pa, :, 2:4], AL.add, 1)
        t2 = rpool.tile([128, 36, 2, 16, 2], BF16, name=f"t2{tag}",
                        tag="t2", bufs=1)[:, :4 * S]
        for pa in range(2):          # b: 2 -> 1
            co_tt(t2[:, :, pa], wz[:, :, pa, :, 0], wz[:, :, pa, :, 1],
                  AL.add, 1)
        t2v = t2.rearrange("p si pa (co a) tp -> p (si pa) co a tp", a=4)
        t3 = rpool.tile([128, 72, 4, 2, 2], BF16, name=f"t3{tag}",
                        tag="t3", bufs=1)[:, :8 * S]
        nc.vector.tensor_tensor(t3, t2v[:, :, :, 0:2], t2v[:, :, :, 2:4],
                                op=AL.add)
        ar = rpool.tile([128, 72, 4, 2], BF16, name=f"ar{tag}", tag="ar",
                        bufs=2)[:, :8 * S]
        nc.vector.tensor_tensor(ar, t3[:, :, :, 0], t3[:, :, :, 1],
                                op=AL.add)
        return ar

    def arsum(ar, r, tag):
        """n2sum[s,co,tp] = sum_i r_i * ar_app[s,i,co,tp] (r=None -> r=1)."""
        av = ar.rearrange("p (s i pa) co tp -> p s i pa co tp",
                          i=4, pa=2)[:, :, :, 1]
        if r is not None:
            w8 = mpool.tile([128, 9, T0, 4, 2], BF16, name=f"w8{tag}",
                            tag="ars_w")[:, :S]
            rv = r.rearrange("p s i (co tp) -> p s i co tp", tp=2)
            nc.vector.tensor_tensor(w8, av, rv, op=AL.mult)
            av = w8
        u1 = mpool.tile([128, 9, 2, 4, 2], F32, name=f"u1{tag}",
                        tag="ars_u")[:, :S]
        nc.vector.tensor_tensor(u1[:, :, 0], av[:, :, 0], av[:, :, 1],
                                op=AL.add)
        nc.vector.tensor_tensor(u1[:, :, 1], av[:, :, 2], av[:, :, 3],
                                op=AL.add)
        n2 = mpool.tile([128, 9, 4, 2], F32, name=f"n2{tag}",
                        tag="st_n2")[:, :S]
        nc.vector.tensor_tensor(n2, u1[:, :, 0], u1[:, :, 1], op=AL.add)
        return n2

    def mult_w_by_p():
        pb = pf.unsqueeze(2).broadcast_to([128, S, T0, 256])
        co_tt(w, Uf, pb, AL.mult, 1)

    def mult_w_by_r(r):
        # r [128, S, T0, 8(co,tp)] -> broadcast over (pa, ab)
        rv = r.rearrange("p s i (co tp) -> p (s i) co tp", tp=2)
        rb = rv.unsqueeze(2).unsqueeze(4).broadcast_to(
            [128, S * 4, 2, 4, 16, 2])
        ub = Uf.rearrange("p s i (pa co ab tp) -> p (s i) pa co ab tp",
                          pa=2, co=4, tp=2)
        wv = wpa.rearrange("p si pa (co ab tp) -> p si pa co ab tp",
                           co=4, tp=2)
        for pa in range(2):
            co_tt(wv[:, :, pa], ub[:, :, pa], rb[:, :, pa], AL.mult, 1)

    def sum_w_into_p():
        co_tt(ts1, w[:, :, 0], w[:, :, 1], AL.add, 1)
        co_tt(ts2, w[:, :, 2], w[:, :, 3], AL.add, 1)
        co_tt(pf, ts1, ts2, AL.add, 1)

    def routstep(ar, sfp, sfa, bacc, first, tag):
        arv = ar.rearrange("p (s i pa) co tp -> p s i pa (co tp)",
                           i=4, pa=2)
        ta = mpool.tile([128, 9, T0, 8], BF16, name=f"ta{tag}",
                        tag="rt_ta")[:, :S]
        tb = mpool.tile([128, 9, T0, 8], BF16, name=f"tb{tag}",
                        tag="rt_tb")[:, :S]
        sfpb = sfp.unsqueeze(2).broadcast_to([128, S, T0, 8])
        sfab = sfa.unsqueeze(2).broadcast_to([128, S, T0, 8])
        nc.vector.tensor_tensor(ta, arv[:, :, :, 0], sfpb, op=AL.mult)
        nc.vector.tensor_tensor(tb, arv[:, :, :, 1], sfab, op=AL.mult)
        if first:
            nc.vector.tensor_tensor(bacc, ta, tb, op=AL.mult)
        else:
            nc.vector.tensor_tensor(ta, ta, tb, op=AL.mult)
            nc.vector.tensor_tensor(bacc, bacc, ta, op=AL.add)

    def n2_from_p(tag):
        sq = mpool.tile([128, 9, 128], BF16, name=f"sq{tag}", tag="st_sq",
                        bufs=1)[:, :S]
        nc.vector.tensor_tensor(sq, p[:, :, 1], p[:, :, 1], op=AL.mult)
        sqv = sq.rearrange("p s (co a b tp) -> p s co tp (a b)",
                           co=4, a=4, b=4)
        n2 = mpool.tile([128, 9, 4, 2], F32, name=f"n2{tag}",
                        tag="st_n2")[:, :S]
        nc.vector.tensor_reduce(n2, sqv, axis=AX.X, op=AL.add)
        return n2

    def sigmoid_act(name):
        r = rpool.tile([128, 9, T0, 8], BF16, name=name, tag="r2", bufs=2)
        nc.scalar.activation(r[:, :S], bacc, AF.Sigmoid)
        return r[:, :S]

    # ---- iter 1 (r = 0.5 folded into scalings) -------------------------
    sfp1 = statP("1")
    sfa1 = statA(n2_from_p("1"), 0.25, 0.5, "1")
    mult_w_by_p()
    ar1 = araw("r1")
    bacc = rpool.tile([128, 9, T0, 8], F32, name="bacc", tag="bacc",
                      bufs=2)[:, :S]
    routstep(ar1, sfp1, sfa1, bacc, True, "r1")

    # ---- iter 2 --------------------------------------------------------
    r2 = sigmoid_act("r2")
    mult_w_by_r(r2)
    sum_w_into_p()
    sfp2 = statP("2")
    sfa2 = statA(n2_from_p("2"), 1.0, 1.0, "2")
    mult_w_by_p()
    ar2 = araw("r2")
    routstep(ar2, sfp2, sfa2, bacc, False, "r2")

    # ---- final ---------------------------------------------------------
    cR = sigmoid_act("cR")
    mult_w_by_r(cR)
    sum_w_into_p()
    sfp3 = statP("3")
    sfa3 = statA(n2_from_p("3"), 1.0, 1.0, "3")
    v3 = rpool.tile([128, 9, 2, 128], BF16, name="v3", tag="v3", bufs=1)
    vv = v3[:, :S].rearrange("p s pa (co ab tp) -> p s pa co ab tp",
                             co=4, tp=2)
    pv = p.rearrange("p s pa (co ab tp) -> p s pa co ab tp", co=4, tp=2)
    sfp3b = sfp3.rearrange("p s (co tp) -> p s co tp", tp=2) \
        .unsqueeze(3).broadcast_to([128, S, 4, 16, 2])
    sfa3b = sfa3.rearrange("p s (co tp) -> p s co tp", tp=2) \
        .unsqueeze(3).broadcast_to([128, S, 4, 16, 2])
    nc.vector.tensor_tensor(vv[:, :, 0], pv[:, :, 0], sfp3b, op=AL.mult)
    nc.vector.tensor_tensor(vv[:, :, 1], pv[:, :, 1], sfa3b, op=AL.mult)
    return v3


def _build_nc():
    nc = bacc.Bacc(None)
    P_d = nc.dram_tensor("patches", [96, T0, NBLK, 4, Z], BF16, kind="ExternalInput")
    CW_d = nc.dram_tensor("convw", [96, T0, 32, 8], BF16, kind="ExternalInput")
    TW_d = nc.dram_tensor("tw", [128, T0, 2, 128], BF16, kind="ExternalInput")
    RW_d = nc.dram_tensor("rw", [128, 4, 32], BF16, kind="ExternalInput")
    KA_d = nc.dram_tensor("ka", [128, T0], F32, kind="ExternalInput")
    XY_d = nc.dram_tensor("xy", [128, NBLK, 2, 32], BF16, kind="ExternalInput")
    ID_d = nc.dram_tensor("ident", [128, 128], BF16, kind="ExternalInput")
    OUT_d = nc.dram_tensor("out", [256, NBLK, 128], BF16, kind="ExternalOutput")

    with tile.TileContext(nc) as tc:
        with (
            tc.tile_pool(name="const", bufs=1) as cpool,
            tc.tile_pool(name="pload", bufs=2) as ppool,
            tc.tile_pool(name="xbuf", bufs=2) as xpool,
            tc.tile_pool(name="stage", bufs=2) as spool,
            tc.tile_pool(name="ubig", bufs=1) as upool,
            tc.tile_pool(name="rscr", bufs=1) as rpool,
            tc.tile_pool(name="small", bufs=2) as mpool,
            tc.tile_pool(name="ps_cv", bufs=2, space="PSUM") as psx,
            tc.tile_pool(name="ps_uh", bufs=3, space="PSUM") as psuh,
            tc.tile_pool(name="ps_ob", bufs=1, space="PSUM") as pso,
        ):
            cw = cpool.tile([96, T0, 32, 8], BF16, name="cw")
            nc.sync.dma_start(cw[:], CW_d[:])
            tw = cpool.tile([128, T0, 2, 128], BF16, name="tw")
            nc.sync.dma_start(tw[:], TW_d[:])
            rw3 = cpool.tile([128, 4, 32], BF16, name="rw3")
            nc.sync.dma_start(rw3[:], RW_d[:])
            ka = cpool.tile([128, T0], F32, name="ka")
            nc.sync.dma_start(ka[:], KA_d[:])
            xy = cpool.tile([128, NBLK, 2, 32], BF16, name="xy")
            nc.sync.dma_start(xy[:], XY_d[:])
            ident = cpool.tile([128, 128], BF16, name="ident")
            nc.sync.dma_start(ident[:], ID_d[:])

            ubig = upool.tile([128, NBLK, T0, 2, 128], BF16, name="ubig")

            for s0, S in CHUNKS:
                rawt = spool.tile([128, 9, T0, 32], BF16, name="rawt",
                                  tag="rawt", bufs=2)
                xsbs = {}

                def conv_stage(i, s0=s0, S=S):
                    pt = ppool.tile([96, 9, 4, Z], BF16, name="pt", tag="pt")
                    nc.sync.dma_start(pt[:, :S], P_d[:, i, s0:s0 + S])
                    # conv: S data-stationary matmuls, 3-row-batched drains
                    xsb = xpool.tile([128, 9, 32, 8], BF16, name="xsb", tag="xsb")
                    for sg in range(S // 3):
                        cps = psx.tile([128, 3, 256], F32, name="cps", tag="cps")
                        for k in range(3):
                            nc.tensor.matmul(
                                cps[:, k],
                                pt[:, 3 * sg + k].rearrange("p j z -> p (j z)"),
                                cw[:, i].rearrange("p f c -> p (f c)"),
                                start=True, stop=True)
                        nc.scalar.copy(
                            xsb[:, 3 * sg:3 * sg + 3].rearrange(
                                "p s f c -> p (s f c)"),
                            cps[:].rearrange("p s f -> p (s f)"))
                    xsbs[i] = xsb

                def transform_stage(i, s0=s0, S=S):
                    xsb = xsbs[i]
                    # transform pos/app: per (pa, rp) psum [128, S, 32]
                    stp = spool.tile([128, 9, 4, 32], BF16, name="stp", tag="stp")
                    sta = spool.tile([128, 9, 4, 32], BF16, name="sta", tag="sta")
                    for pa, stg_t in ((0, stp), (1, sta)):
                        for rp in range(4):
                            ups = psuh.tile([128, 9, 32], F32, name="ups", tag="uh")
                            for cp in range(4):
                                nc.tensor.matmul(
                                    ups[32 * cp:32 * cp + 32, :S],
                                    tw[32 * rp:32 * rp + 32, i, pa,
                                       32 * cp:32 * cp + 32],
                                    xsb[32 * rp:32 * rp + 32, :S, :, pa * 4 + cp],
                                    start=True, stop=True,
                                    tile_position=(32 * rp, 32 * cp))
                            if pa == 0:
                                nc.scalar.copy(stg_t[:, :S, rp, :], ups[:, :S])
                            else:
                                nc.scalar.activation(
                                    stg_t[:, :S, rp, :], ups[:, :S],
                                    AF.Identity, bias=ka[:, i:i + 1])
                    # raw extraction (pos conv channels, c=3 picks), m''=(co,a,tp)
                    rstg = spool.tile([32, 9, 4, 32], BF16, name="rstg", tag="rstg")
                    for rp in range(4):
                        rfull = psuh.tile([128, 9, 32], F32, name="rps", tag="uh")
                        rps = rfull[0:32, :S]
                        for cp in range(4):
                            nc.tensor.matmul(
                                rps,
                                rw3[32 * rp:32 * rp + 32, cp, :],
                                xsb[32 * rp:32 * rp + 32, :S, :, cp],
                                start=(cp == 0), stop=(cp == 3),
                                tile_position=(32 * rp, 0))
                        nc.scalar.copy(rstg[:, :S, rp, :], rps)
                    # batched DMA-transposes to pixel-major layouts
                    nc.sync.dma_start(
                        ubig[:, s0:s0 + S, i, 0, :],
                        stp[:, :S].rearrange("p s j f -> p (s j f)"),
                        transpose=True)
                    nc.sync.dma_start(
                        ubig[:, s0:s0 + S, i, 1, :],
                        sta[:, :S].rearrange("p s j f -> p (s j f)"),
                        transpose=True)
                    nc.sync.dma_start(
                        rawt[:, :S, i, :],
                        rstg[:, :S].rearrange("p s j f -> p (s j f)"),
                        transpose=True)

                # software pipeline: conv(i+1) is emitted before transform(i)
                # so the PE has runway while ACT drains conv psum
                conv_stage(0)
                for i in range(T0):
                    if i + 1 < T0:
                        conv_stage(i + 1)
                    transform_stage(i)

                # routing for this chunk
                U = ubig[:, s0:s0 + S]
                v3 = _routing_chunk(nc, mpool, rpool, U, rawt[:, :S], xy, s0, S)

                # output: PE transpose + batched drains + one DMA per chunk
                osb = mpool.tile([128, 2, 9, 128], BF16, name="osb", tag="osb",
                                 bufs=1)
                for g in range(S // 3):
                    ops = pso.tile([128, 6, 128], BF16, name="ops", tag="ob")
                    for k in range(6):
                        sp = 6 * g + k
                        s, pa = sp // 2, sp % 2
                        nc.tensor.transpose(ops[:, k], v3[:, s, pa], ident[:])
                    nc.scalar.copy(
                        osb[:, :, 3 * g:3 * g + 3, :],
                        ops[:].rearrange("p (s pa) f -> p pa s f", pa=2))
                nc.sync.dma_start(
                    OUT_d.rearrange("(pa c) b f -> c pa b f", pa=2)[:, :, s0:s0 + S],
                    osb[:, :, :S])
    nc.finalize()
    return nc


_NC_CACHE = None


def _get_nc():
    global _NC_CACHE
    if _NC_CACHE is None:
        _NC_CACHE = _build_nc()
    return _NC_CACHE


def kernel(input_tensor, W_conv, W_pos, W_app, b_app):
    input_tensor = np.asarray(input_tensor, np.float32)
    CW, TW, RW3, KA = _build_weights(np.asarray(W_conv, np.float32),
                                     np.asarray(W_pos, np.float32),
                                     np.asarray(W_app, np.float32),
                                     np.asarray(b_app, np.float32))
    N = input_tensor.shape[0]
    full_pad = np.pad(input_tensor, ((0, 0), (0, 0), (0, 0), (2, 2), (2, 2)))
    bf = np.float16
    ident = np.eye(128, dtype=np.float32)
    in_maps = []
    for c in range(8):
        n, hh = c // 2, c % 2
        sl = full_pad[n, :, :, 48 * hh:48 * hh + 52, :]
        in_maps.append({
            "patches": _build_patches(sl).astype(bf),
            "convw": CW.astype(bf),
            "tw": TW.astype(bf),
            "rw": RW3.astype(bf),
            "ka": KA.astype(np.float32),
            "xy": _pixel_coords(hh).astype(bf),
            "ident": ident.astype(bf),
        })
    nc = _get_nc()
    kres = run_bass_kernel_spmd(nc, in_maps, core_ids=list(range(8)))
    global LAST_RESULT
    LAST_RESULT = kres
    res = kres.results
    # unscramble: out dram [256=(pa,co,a,b,tp), blk36, px128=(j,pi,pj)]
    blk = np.arange(NBLK)
    j = np.arange(4)
    pi = np.arange(4)
    pj = np.arange(8)
    hmap = (4 * (blk // 3))[:, None, None, None] + pi[None, None, :, None]
    hmap = np.broadcast_to(hmap, (NBLK, 4, 4, 8)).ravel()
    wmap = (32 * (blk % 3))[:, None, None, None] + 8 * j[None, :, None, None] \
        + pj[None, None, None, :]
    wmap = np.broadcast_to(wmap, (NBLK, 4, 4, 8)).ravel()
    out = np.zeros((N, T1, Z, H, W), np.float32)
    for c in range(8):
        n, hh = c // 2, c % 2
        img = np.zeros((256, HC, W), np.float32)
        img[:, hmap, wmap] = res[c]["out"].astype(np.float32).reshape(256, NBLK * 128)
        o = img.reshape(2, 4, 4, 4, 2, HC, W)   # [pa, co, a, b, tp]
        for pa in range(2):
            for co in range(4):
                for tp in range(2):
                    t1 = 2 * co + tp
                    zblk = o[pa, co, :, :, tp].reshape(16, HC, W)
                    out[n, t1, pa * 16:pa * 16 + 16,
                        48 * hh:48 * hh + 48] = zblk
    return out
